# revision 36
# baseline (speedup 1.0000x reference)
"""Gaussian kernel matrix (pairwise L2 over T) for x:(32,64,1000,16) -> (32,64,64,16).

out[n,c,d,f] = exp(-||x[n,c,:,f] - x[n,d,:,f]||^2 / 2)

Strategy (8 NeuronCores, data-parallel over N, 4 batch elems per core):
  Per core, per pair of batch elems (2n x 64c = 128 partitions):
    1. DMA natural-layout slab HBM->SBUF with fp32->bf16 cast, in 5
       chunk-aligned sections (1MB first so PE starts early, 2MB middles for
       SDMA line rate, small last so little work remains after the final
       byte). Identity constants arrive as host inputs over the (otherwise
       idle) sync HWDGE queue, in parallel with the gpsimd SWDGE setup.
    2. Per t-chunk of 128: transpose [128(2n,c), 128t] tiles per f via
       *normal* matmuls (chunk as stationary, identity as moving operand,
       fp32 PSUM) -- unlike is_transpose matmuls these count as PE-busy so
       the HAM clock gate warms to 2.4 GHz. Two 8-f half-chunk PSUM tiles
       per chunk (triple-buffered pool); PSUM->SBUF copies alternate DVE/ACT.
    3. Gram matmuls lag one chunk behind the transposes (copy latency hides
       under the next chunk's matmuls), block-compressed: only the two
       diagonal 64x64 blocks per f (N=64 matmuls, col-group tiled so both
       halves run concurrently on the PE array), accumulated into one
       [128,F/2,64] fp32 PSUM bank per f-group across the 8 chunks. The
       banks are DVE-memset to 0 first and every matmul uses start=False,
       making the accumulation independent of stale has_written bits.
    4. Epilogue, block-compressed at full lane width: sqh = row-reduce of
       G .* (-I64/2); dti = G/2 + sqh (exactly 0 on the diagonal); H =
       exp(dti); O = H .* H^T_block. Diagonal is exactly 1. H^T via small
       64x64 matmuls into recycled transpose-PSUM slots. Pair 0's epilogue
       chain is software-pipelined under pair 1's first transpose chunks
       (whose copies route to ACT while DVE runs the chains).
    5. Output DMA per batch element on the sync HWDGE queue.
bf16 matmul inputs with fp32 PSUM accumulation; the epilogue's exact diagonal
cancellation makes the output independent of the bf16 rounding on-diagonal.
"""

import numpy as np

N_FULL, C, T, F = 32, 64, 1000, 16
N_CORES = 8
N_PER_CORE = N_FULL // N_CORES  # 4
NPAIRS = N_PER_CORE // 2        # 2
TPAD = 1024
TCH = TPAD // 128               # 8 t-chunks
FG = 2                          # f-groups
F_PER_G = F // FG               # 8

_CACHE = {}


def _split_multi_waits(bir_bytes):
    """Walrus codegen here only supports one sync-wait per instruction; Tile
    emits several. Split extras into preceding NoOp instructions on the same
    engine queue (engine executes in order, so the waits still gate)."""
    import json

    bir = json.loads(bir_bytes)
    cnt = 0
    for fn in bir["functions"]:
        for blk in fn["blocks"]:
            new = []
            for inst in blk["instructions"]:
                si = inst.get("sync_info")
                waits = (si or {}).get("on_wait", [])
                if len(waits) > 1:
                    for w in waits[:-1]:
                        cnt += 1
                        new.append(
                            {
                                "debug": inst.get("debug", 0),
                                "engine": inst["engine"],
                                "ins": [],
                                "outs": [],
                                "name": f"WS{cnt}",
                                "opcode": "NoOp",
                                "sync_info": {"on_update": [], "on_wait": [w]},
                            }
                        )
                    si["on_wait"] = waits[-1:]
                new.append(inst)
            blk["instructions"] = new
    return json.dumps(bir).encode()


def _build_nc():
    import concourse.bass as bass
    import concourse.mybir as mybir
    import concourse.tile as tile

    dt = mybir.dt
    nc = bass.Bass(enable_partition_id=False)
    x = nc.dram_tensor("x", (N_PER_CORE, C, T, F), dt.float32, kind="ExternalInput")
    identb_d = nc.dram_tensor("identb", (128, 128), dt.bfloat16, kind="ExternalInput")
    identm_d = nc.dram_tensor("identm", (128, 64), dt.float32, kind="ExternalInput")
    y = nc.dram_tensor("y", (N_PER_CORE, C, C, F), dt.float32, kind="ExternalOutput")

    with tile.TileContext(nc) as tc:
        with (
            tc.tile_pool(name="const", bufs=1) as constp,
            tc.tile_pool(name="slab", bufs=2) as slabp,
            tc.tile_pool(name="trT", bufs=3) as trp,
            tc.tile_pool(name="work", bufs=2) as workp,
            tc.tile_pool(name="osb", bufs=2) as outp,
            tc.tile_pool(name="ps_tr", bufs=3, space="PSUM") as ps_tr,
            tc.tile_pool(name="ps_gram", bufs=1, space="PSUM") as ps_gram,
        ):
            # identity constants over the sync HWDGE queue (parallel to the
            # gpsimd SWDGE input stream)
            ident_bf = constp.tile([128, 128], dt.bfloat16)
            identm = constp.tile([128, 64], dt.float32)  # -0.5 * I64, both halves
            nc.sync.dma_start(ident_bf, identb_d[:, :])
            nc.sync.dma_start(identm, identm_d[:, :])

            # chunk-aligned sections: small first (early PE start), small
            # last (short post-DMA tail); 2MB middles keep SDMA at line rate
            TSEC = [(0, 128), (128, 384), (384, 640), (640, 896), (896, 1000)]

            def emit_dma(p):
                slab = slabp.tile([128, TPAD, F], dt.bfloat16, tag="slab")
                src = x[2 * p : 2 * p + 2].rearrange("n c t f -> (n c) t f")
                for lo, hi in TSEC:
                    nc.gpsimd.dma_start(
                        slab[:, lo:hi, :], src[:, lo:hi, :]
                    )  # fp32 -> bf16 cast
                return slab

            slabs = [emit_dma(p) for p in range(NPAIRS)]
            for slab in slabs:
                nc.gpsimd.memset(slab[:, T:, :], 0.0)

            def emit_fill(trps, slab, n):
                """HAM-warming fillers: N=512 matmuls that keep the PE array
                busy while the next DMA section lands. They write the first
                PSUM bank of `trps`, which the real transposes that follow
                overwrite (start=True) before anything reads it."""
                for _ in range(n):
                    nc.tensor.matmul(
                        trps[:, 0:4, :], ident_bf, slab[:, 0:32, :]
                    )

            def emit_T(slab, ch, copies_on_act=False, fillers=0):
                """Transpose one t-chunk (all 16 f) via normal matmuls.
                copies_on_act routes both PSUM->SBUF copies to ACT so they
                don't queue behind epilogue chains on DVE."""
                trT_ch = trp.tile([128, F, 128], dt.bfloat16, tag="trT")
                for g in range(FG):
                    trps = ps_tr.tile([128, F_PER_G, 128], dt.float32, tag="trps")
                    if g == 0 and fillers:
                        emit_fill(trps, slab, fillers)
                    for j in range(F_PER_G):
                        f = g * F_PER_G + j
                        nc.tensor.matmul(
                            trps[:, j, :],
                            slab[:, ch * 128 : (ch + 1) * 128, f],
                            ident_bf,
                        )
                    dstg = trT_ch[:, g * F_PER_G : (g + 1) * F_PER_G, :]
                    if g == 0 and not copies_on_act:
                        nc.vector.tensor_copy(dstg, trps)
                    else:
                        nc.scalar.copy(dstg, trps)
                return trT_ch

            def emit_G(grams, trT_ch, ch, fs=range(F)):
                # block-compressed grams: only the two diagonal 64x64 blocks
                # per f are computed (N=64, col-group tiled -> concurrent).
                # PSUM was DVE-memset to 0, so accumulate-vs-overwrite from
                # stale has_written bits is correct either way; no start
                # flag gymnastics needed.
                for f in fs:
                    g, f8 = f // F_PER_G, f % F_PER_G
                    for m in range(2):
                        sl = slice(64 * m, 64 * m + 64)
                        nc.tensor.matmul(
                            grams[g][sl, f8, :],
                            trT_ch[:, f, sl],
                            trT_ch[:, f, sl],
                            start=False,
                            stop=(ch == TCH - 1),
                            skip_group_check=True,
                        )

            def emit_E_chain(grams, g, work):
                """Epilogue chain for f-group g -> h [128,8,128] (full width;
                cross-block entries of h underflow to 0 and are never read)."""
                Gg = grams[g]
                masked = workp.tile([128, F_PER_G, 64], dt.float32, tag=f"masked{g}")
                sqh = workp.tile([128, F_PER_G], dt.float32, tag=f"sqh{g}")
                dti = workp.tile([128, F_PER_G, 64], dt.float32, tag=f"dti{g}")
                h = workp.tile([128, F_PER_G, 64], dt.bfloat16, tag=f"h{g}")
                # everything block-compressed [128, 8, 64] at full lane width
                nc.vector.tensor_tensor(
                    masked,
                    Gg,
                    identm[:, None, :].to_broadcast((128, F_PER_G, 64)),
                    mybir.AluOpType.mult,
                )
                nc.vector.reduce_sum(sqh, masked, axis=mybir.AxisListType.X)
                # dti = G/2 + sqh  (= (G - diag)/2 on the block diagonal)
                nc.vector.scalar_tensor_tensor(
                    dti,
                    Gg,
                    0.5,
                    sqh[:, :, None].to_broadcast((128, F_PER_G, 64)),
                    mybir.AluOpType.mult,
                    mybir.AluOpType.add,
                )
                nc.scalar.activation(h, dti, mybir.ActivationFunctionType.Exp)
                work[g] = h

            def emit_E_tt(g, work):
                h = work[g]
                tt = ps_tr.tile([128, F_PER_G, 64], dt.float32, tag="trps")
                for m in range(2):
                    sl = slice(64 * m, 64 * m + 64)
                    for f8 in range(F_PER_G):
                        nc.tensor.matmul(
                            tt[sl, f8, :], h[sl, f8, :], ident_bf[sl, sl]
                        )
                work[g, "tt"] = tt

            def emit_E_out(p, work, out_sb, g):
                h, tt = work[g], work[g, "tt"]
                nc.vector.tensor_tensor(
                    out_sb[:, :, g * F_PER_G : (g + 1) * F_PER_G].rearrange(
                        "p d f -> p f d"
                    ),
                    h,
                    tt,
                    mybir.AluOpType.mult,
                )
                if g == FG - 1:
                    nc.sync.dma_start(y[2 * p], out_sb[0:64])
                    nc.sync.dma_start(y[2 * p + 1], out_sb[64:128])

            def new_grams():
                out = []
                for g in range(FG):
                    gt = ps_gram.tile(
                        [128, F_PER_G, 64], dt.float32, tag=f"gram{g}", name=f"gram{g}"
                    )
                    nc.vector.memset(gt, 0.0)
                    out.append(gt)
                return out

            # ---- pair 0 ----
            grams0 = new_grams()
            trT = {}
            trT[0] = emit_T(slabs[0], 0)
            for ch in range(1, TCH - 1):
                trT[ch] = emit_T(slabs[0], ch, fillers=3 if ch % 2 == 1 else 0)
                emit_G(grams0, trT.pop(ch - 1), ch - 1)
            emit_G(grams0, trT.pop(TCH - 2), TCH - 2)  # G[6] before T[7]
            trT[7] = emit_T(slabs[0], TCH - 1, fillers=3)
            emit_G(grams0, trT[7], TCH - 1, fs=range(0, F_PER_G))
            work0 = {}
            emit_E_chain(grams0, 0, work0)
            emit_G(grams0, trT.pop(7), TCH - 1, fs=range(F_PER_G, F))
            emit_E_chain(grams0, 1, work0)
            # pair 1's first chunks (copies on ACT -- DVE is busy with pair
            # 0's chains) interleave with pair 0's epilogue PE work, ordered
            # by DMA section arrival so nothing ready queues behind a wait
            grams1 = new_grams()
            out_sb0 = outp.tile([128, C, F], dt.float32, tag="osb")
            trT[8] = emit_T(slabs[1], 0, copies_on_act=True)
            emit_G(grams1, trT.pop(8), 0)
            emit_E_tt(0, work0)
            trT[9] = emit_T(slabs[1], 1, copies_on_act=True, fillers=3)
            trT[10] = emit_T(slabs[1], 2, copies_on_act=True)
            emit_G(grams1, trT.pop(9), 1)
            emit_E_tt(1, work0)
            emit_E_out(0, work0, out_sb0, 0)
            trT[11] = emit_T(slabs[1], 3, copies_on_act=True, fillers=3)
            emit_G(grams1, trT.pop(10), 2)
            emit_E_out(0, work0, out_sb0, 1)
            trT[12] = emit_T(slabs[1], 4)
            emit_G(grams1, trT.pop(11), 3)

            # ---- pair 1 (remaining chunks) ----
            for ch in range(5, TCH - 1):
                trT[8 + ch] = emit_T(slabs[1], ch, fillers=3 if ch == 5 else 0)
                emit_G(grams1, trT.pop(8 + ch - 1), ch - 1)
            emit_G(grams1, trT.pop(8 + TCH - 2), TCH - 2)  # G[6] before T[7]
            trT[15] = emit_T(slabs[1], TCH - 1, fillers=3)
            emit_G(grams1, trT[15], TCH - 1, fs=range(0, F_PER_G))
            work1 = {}
            emit_E_chain(grams1, 0, work1)
            emit_G(grams1, trT.pop(15), TCH - 1, fs=range(F_PER_G, F))
            emit_E_chain(grams1, 1, work1)
            out_sb1 = outp.tile([128, C, F], dt.float32, tag="osb")
            ftt = ps_tr.tile([128, F_PER_G, 128], dt.float32, tag="trps", name="ftt")
            emit_fill(ftt, slabs[1], 8)
            emit_E_tt(0, work1)
            emit_E_tt(1, work1)
            emit_E_out(1, work1, out_sb1, 0)
            emit_E_out(1, work1, out_sb1, 1)

    orig_ser = nc.to_json_bytes
    nc.to_json_bytes = lambda: _split_multi_waits(orig_ser())
    return nc


def _get_nc():
    if "nc" not in _CACHE:
        _CACHE["nc"] = _build_nc()
    return _CACHE["nc"]


def _const_inputs():
    import ml_dtypes

    identb = np.eye(128, dtype=np.float32).astype(ml_dtypes.bfloat16)
    identm = np.vstack([-0.5 * np.eye(64), -0.5 * np.eye(64)]).astype(np.float32)
    return identb, identm


def kernel(x, _trace=False):
    from concourse.bass_utils import run_bass_kernel_spmd

    x = np.ascontiguousarray(np.asarray(x), dtype=np.float32)
    assert x.shape == (N_FULL, C, T, F), x.shape
    nc = _get_nc()
    identb, identm = _const_inputs()
    in_maps = [
        {
            "x": np.ascontiguousarray(x[N_PER_CORE * i : N_PER_CORE * (i + 1)]),
            "identb": identb,
            "identm": identm,
        }
        for i in range(N_CORES)
    ]
    res = run_bass_kernel_spmd(nc, in_maps, core_ids=list(range(N_CORES)), trace=_trace)
    out = np.concatenate([r["y"] for r in res.results], axis=0)
    if _trace:
        _CACHE["last_result"] = res
    return out


# revision 38
# speedup vs baseline: 1.0516x; 1.0516x over previous
"""Gaussian kernel matrix (pairwise L2 over T) for x:(32,64,1000,16) -> (32,64,64,16).

out[n,c,d,f] = exp(-||x[n,c,:,f] - x[n,d,:,f]||^2 / 2)

Strategy (8 NeuronCores, data-parallel over N, 4 batch elems per core):
  Per core, per pair of batch elems (2n x 64c = 128 partitions):
    1. DMA natural-layout slab HBM->SBUF with fp32->bf16 cast, in 5
       chunk-aligned sections (1MB first so PE starts early, 2MB middles for
       SDMA line rate, small last so little work remains after the final
       byte). Identity constants arrive as host inputs over the (otherwise
       idle) sync HWDGE queue, in parallel with the gpsimd SWDGE setup.
    2. Per t-chunk of 128: transpose [128(2n,c), 128t] tiles per f via
       *normal* matmuls (chunk as stationary, identity as moving operand,
       fp32 PSUM) -- unlike is_transpose matmuls these count as PE-busy so
       the HAM clock gate warms to 2.4 GHz. Two 8-f half-chunk PSUM tiles
       per chunk (triple-buffered pool); PSUM->SBUF copies alternate DVE/ACT.
    3. Gram matmuls lag one chunk behind the transposes (copy latency hides
       under the next chunk's matmuls), block-compressed: only the two
       diagonal 64x64 blocks per f (N=64 matmuls, col-group tiled so both
       halves run concurrently on the PE array), accumulated into one
       [128,F/2,64] fp32 PSUM bank per f-group across the 8 chunks. The
       banks are DVE-memset to 0 first and every matmul uses start=False,
       making the accumulation independent of stale has_written bits.
    4. Epilogue, block-compressed at full lane width: sqh = row-reduce of
       G .* (-I64/2); dti = G/2 + sqh (exactly 0 on the diagonal); H =
       exp(dti); O = H .* H^T_block. Diagonal is exactly 1. H^T via small
       64x64 matmuls into recycled transpose-PSUM slots. Pair 0's epilogue
       chain is software-pipelined under pair 1's first transpose chunks
       (whose copies route to ACT while DVE runs the chains).
    5. Output DMA per batch element on the sync HWDGE queue.
bf16 matmul inputs with fp32 PSUM accumulation; the epilogue's exact diagonal
cancellation makes the output independent of the bf16 rounding on-diagonal.
"""

import numpy as np

N_FULL, C, T, F = 32, 64, 1000, 16
N_CORES = 8
N_PER_CORE = N_FULL // N_CORES  # 4
NPAIRS = N_PER_CORE // 2        # 2
TPAD = 1024
TCH = TPAD // 128               # 8 t-chunks
FG = 2                          # f-groups
F_PER_G = F // FG               # 8

_CACHE = {}


def _split_multi_waits(bir_bytes):
    """Walrus codegen here only supports one sync-wait per instruction; Tile
    emits several. Split extras into preceding NoOp instructions on the same
    engine queue (engine executes in order, so the waits still gate)."""
    import json

    bir = json.loads(bir_bytes)
    cnt = 0
    for fn in bir["functions"]:
        for blk in fn["blocks"]:
            new = []
            for inst in blk["instructions"]:
                si = inst.get("sync_info")
                waits = (si or {}).get("on_wait", [])
                if len(waits) > 1:
                    for w in waits[:-1]:
                        cnt += 1
                        new.append(
                            {
                                "debug": inst.get("debug", 0),
                                "engine": inst["engine"],
                                "ins": [],
                                "outs": [],
                                "name": f"WS{cnt}",
                                "opcode": "NoOp",
                                "sync_info": {"on_update": [], "on_wait": [w]},
                            }
                        )
                    si["on_wait"] = waits[-1:]
                new.append(inst)
            blk["instructions"] = new
    return json.dumps(bir).encode()


def _build_nc():
    import concourse.bass as bass
    import concourse.mybir as mybir
    import concourse.tile as tile

    dt = mybir.dt
    nc = bass.Bass(enable_partition_id=False)
    x = nc.dram_tensor("x", (N_PER_CORE, C, T, F), dt.float32, kind="ExternalInput")
    identb_d = nc.dram_tensor("identb", (128, 128), dt.bfloat16, kind="ExternalInput")
    identm_d = nc.dram_tensor("identm", (128, 64), dt.float32, kind="ExternalInput")
    y = nc.dram_tensor("y", (N_PER_CORE, C, C, F), dt.float32, kind="ExternalOutput")

    with tile.TileContext(nc) as tc:
        with (
            tc.tile_pool(name="const", bufs=1) as constp,
            tc.tile_pool(name="slab", bufs=2) as slabp,
            tc.tile_pool(name="trT", bufs=3) as trp,
            tc.tile_pool(name="work", bufs=2) as workp,
            tc.tile_pool(name="osb", bufs=2) as outp,
            tc.tile_pool(name="ps_tr", bufs=3, space="PSUM") as ps_tr,
            tc.tile_pool(name="ps_gram", bufs=1, space="PSUM") as ps_gram,
        ):
            # identity constants over the sync HWDGE queue (parallel to the
            # gpsimd SWDGE input stream)
            ident_bf = constp.tile([128, 128], dt.bfloat16)
            identm = constp.tile([128, 64], dt.float32)  # -0.5 * I64, both halves
            nc.sync.dma_start(ident_bf, identb_d[:, :])
            nc.sync.dma_start(identm, identm_d[:, :])

            # chunk-aligned sections: 1MB early ones so the first chunks
            # gate finely while PE ramps, 2MB middles for SDMA line rate,
            # small last so little work remains after the final byte
            TSEC = [
                (0, 128),
                (128, 256),
                (256, 512),
                (512, 768),
                (768, 896),
                (896, 1000),
            ]

            def emit_dma(p):
                slab = slabp.tile([128, TPAD, F], dt.bfloat16, tag="slab")
                src = x[2 * p : 2 * p + 2].rearrange("n c t f -> (n c) t f")
                for lo, hi in TSEC:
                    nc.gpsimd.dma_start(
                        slab[:, lo:hi, :], src[:, lo:hi, :]
                    )  # fp32 -> bf16 cast
                return slab

            slabs = [emit_dma(p) for p in range(NPAIRS)]
            for slab in slabs:
                nc.gpsimd.memset(slab[:, T:, :], 0.0)

            def emit_T(slab, ch, copies_on_act=False):
                """Transpose one t-chunk (all 16 f) via normal matmuls.
                copies_on_act routes both PSUM->SBUF copies to ACT so they
                don't queue behind epilogue chains on DVE."""
                trT_ch = trp.tile([128, F, 128], dt.bfloat16, tag="trT")
                for g in range(FG):
                    trps = ps_tr.tile([128, F_PER_G, 128], dt.float32, tag="trps")
                    for j in range(F_PER_G):
                        f = g * F_PER_G + j
                        nc.tensor.matmul(
                            trps[:, j, :],
                            slab[:, ch * 128 : (ch + 1) * 128, f],
                            ident_bf,
                        )
                    dstg = trT_ch[:, g * F_PER_G : (g + 1) * F_PER_G, :]
                    if g == 0 and not copies_on_act:
                        nc.vector.tensor_copy(dstg, trps)
                    else:
                        nc.scalar.copy(dstg, trps)
                return trT_ch

            def emit_G(grams, trT_ch, ch, fs=range(F)):
                # block-compressed grams: only the two diagonal 64x64 blocks
                # per f are computed (N=64, col-group tiled -> concurrent).
                # PSUM was DVE-memset to 0, so accumulate-vs-overwrite from
                # stale has_written bits is correct either way; no start
                # flag gymnastics needed.
                for f in fs:
                    g, f8 = f // F_PER_G, f % F_PER_G
                    for m in range(2):
                        sl = slice(64 * m, 64 * m + 64)
                        nc.tensor.matmul(
                            grams[g][sl, f8, :],
                            trT_ch[:, f, sl],
                            trT_ch[:, f, sl],
                            start=False,
                            stop=(ch == TCH - 1),
                            skip_group_check=True,
                        )

            def emit_E_chain(grams, g, work):
                """Epilogue chain for f-group g -> h [128,8,128] (full width;
                cross-block entries of h underflow to 0 and are never read)."""
                Gg = grams[g]
                masked = workp.tile([128, F_PER_G, 64], dt.float32, tag=f"masked{g}")
                sqh = workp.tile([128, F_PER_G], dt.float32, tag=f"sqh{g}")
                dti = workp.tile([128, F_PER_G, 64], dt.float32, tag=f"dti{g}")
                h = workp.tile([128, F_PER_G, 64], dt.bfloat16, tag=f"h{g}")
                # everything block-compressed [128, 8, 64] at full lane width
                nc.vector.tensor_tensor(
                    masked,
                    Gg,
                    identm[:, None, :].to_broadcast((128, F_PER_G, 64)),
                    mybir.AluOpType.mult,
                )
                nc.vector.reduce_sum(sqh, masked, axis=mybir.AxisListType.X)
                # dti = G/2 + sqh  (= (G - diag)/2 on the block diagonal)
                nc.vector.scalar_tensor_tensor(
                    dti,
                    Gg,
                    0.5,
                    sqh[:, :, None].to_broadcast((128, F_PER_G, 64)),
                    mybir.AluOpType.mult,
                    mybir.AluOpType.add,
                )
                # exp per partition half: the first H^T matmuls (m=0) can
                # start as soon as the first half lands
                for m in range(2):
                    sl = slice(64 * m, 64 * m + 64)
                    nc.scalar.activation(
                        h[sl], dti[sl], mybir.ActivationFunctionType.Exp
                    )
                work[g] = h

            def emit_E_tt(g, work):
                h = work[g]
                tt = ps_tr.tile([128, F_PER_G, 64], dt.float32, tag="trps")
                for m in range(2):
                    sl = slice(64 * m, 64 * m + 64)
                    for f8 in range(F_PER_G):
                        nc.tensor.matmul(
                            tt[sl, f8, :], h[sl, f8, :], ident_bf[sl, sl]
                        )
                work[g, "tt"] = tt

            def emit_E_out(p, work, out_sb, g):
                h, tt = work[g], work[g, "tt"]
                nc.vector.tensor_tensor(
                    out_sb[:, :, g * F_PER_G : (g + 1) * F_PER_G].rearrange(
                        "p d f -> p f d"
                    ),
                    h,
                    tt,
                    mybir.AluOpType.mult,
                )
                if g == FG - 1:
                    nc.sync.dma_start(y[2 * p], out_sb[0:64])
                    nc.sync.dma_start(y[2 * p + 1], out_sb[64:128])

            def new_grams():
                out = []
                for g in range(FG):
                    gt = ps_gram.tile(
                        [128, F_PER_G, 64], dt.float32, tag=f"gram{g}", name=f"gram{g}"
                    )
                    nc.vector.memset(gt, 0.0)
                    out.append(gt)
                return out

            # ---- pair 0 ----
            grams0 = new_grams()
            trT = {}
            trT[0] = emit_T(slabs[0], 0)
            for ch in range(1, TCH - 1):
                trT[ch] = emit_T(slabs[0], ch)
                emit_G(grams0, trT.pop(ch - 1), ch - 1)
            emit_G(grams0, trT.pop(TCH - 2), TCH - 2)  # G[6] before T[7]
            trT[7] = emit_T(slabs[0], TCH - 1)
            emit_G(grams0, trT[7], TCH - 1, fs=range(0, F_PER_G))
            work0 = {}
            emit_E_chain(grams0, 0, work0)
            emit_G(grams0, trT.pop(7), TCH - 1, fs=range(F_PER_G, F))
            emit_E_chain(grams0, 1, work0)
            # pair 1's first chunks (copies on ACT -- DVE is busy with pair
            # 0's chains) interleave with pair 0's epilogue PE work, ordered
            # by DMA section arrival so nothing ready queues behind a wait
            grams1 = new_grams()
            out_sb0 = outp.tile([128, C, F], dt.float32, tag="osb")
            trT[8] = emit_T(slabs[1], 0, copies_on_act=True)
            emit_G(grams1, trT.pop(8), 0)
            emit_E_tt(0, work0)
            trT[9] = emit_T(slabs[1], 1, copies_on_act=True)
            trT[10] = emit_T(slabs[1], 2, copies_on_act=True)
            emit_G(grams1, trT.pop(9), 1)
            emit_E_tt(1, work0)
            emit_E_out(0, work0, out_sb0, 0)
            trT[11] = emit_T(slabs[1], 3, copies_on_act=True)
            emit_G(grams1, trT.pop(10), 2)
            emit_E_out(0, work0, out_sb0, 1)
            trT[12] = emit_T(slabs[1], 4)
            emit_G(grams1, trT.pop(11), 3)

            # ---- pair 1 (remaining chunks) ----
            for ch in range(5, TCH - 1):
                trT[8 + ch] = emit_T(slabs[1], ch)
                emit_G(grams1, trT.pop(8 + ch - 1), ch - 1)
            emit_G(grams1, trT.pop(8 + TCH - 2), TCH - 2)  # G[6] before T[7]
            trT[15] = emit_T(slabs[1], TCH - 1)
            emit_G(grams1, trT[15], TCH - 1, fs=range(0, F_PER_G))
            work1 = {}
            emit_E_chain(grams1, 0, work1)
            emit_G(grams1, trT.pop(15), TCH - 1, fs=range(F_PER_G, F))
            emit_E_chain(grams1, 1, work1)
            out_sb1 = outp.tile([128, C, F], dt.float32, tag="osb")
            emit_E_tt(0, work1)
            emit_E_tt(1, work1)
            emit_E_out(1, work1, out_sb1, 0)
            emit_E_out(1, work1, out_sb1, 1)

    orig_ser = nc.to_json_bytes
    nc.to_json_bytes = lambda: _split_multi_waits(orig_ser())
    return nc


def _get_nc():
    if "nc" not in _CACHE:
        _CACHE["nc"] = _build_nc()
    return _CACHE["nc"]


def _const_inputs():
    import ml_dtypes

    identb = np.eye(128, dtype=np.float32).astype(ml_dtypes.bfloat16)
    identm = np.vstack([-0.5 * np.eye(64), -0.5 * np.eye(64)]).astype(np.float32)
    return identb, identm


def kernel(x, _trace=False):
    from concourse.bass_utils import run_bass_kernel_spmd

    x = np.ascontiguousarray(np.asarray(x), dtype=np.float32)
    assert x.shape == (N_FULL, C, T, F), x.shape
    nc = _get_nc()
    identb, identm = _const_inputs()
    in_maps = [
        {
            "x": np.ascontiguousarray(x[N_PER_CORE * i : N_PER_CORE * (i + 1)]),
            "identb": identb,
            "identm": identm,
        }
        for i in range(N_CORES)
    ]
    res = run_bass_kernel_spmd(nc, in_maps, core_ids=list(range(N_CORES)), trace=_trace)
    out = np.concatenate([r["y"] for r in res.results], axis=0)
    if _trace:
        _CACHE["last_result"] = res
    return out
